# revision 17
# baseline (speedup 1.0000x reference)
"""Multi-head attention (B=2, S=2048, D=1024, H=16) on 8 Trainium2 cores.

Sharding: core i -> batch i//4, head-group i%4 (4 heads = 2 pairs of 2).
v4: fp8(e4m3) DoubleRow matmuls everywhere the precision budget allows.
Host pre-scales Wq/Wk/Wv (and bq/bk) by 32 so fp8 values sit in e4m3's
sweet spot; the 1/32^2 is folded into the exp scale and 1/32 into Wo.
 - QKV projections: x and W stream as fp8, K=256 per DoubleRow matmul.
 - Scores: per-head zero-padded DoubleRow (dead half multiplies junk but
   the zeroed weight half kills it), 0.5 cyc/col.
 - exp: split across ACT (exact, fp8 out, bias -2 folded) and DVE
   (Schraudolph int16/bf16 fast-exp) with a Pool bf16->fp8 downcast so
   attn.V stays uniformly fp8 DoubleRow over key-chunk pairs.
 - Z rows via fp8 ones-weight DoubleRow matmuls (ISA limits dual-fp8
   weight tiles to 32/64/128 cols, so no 65th ones-row in vaug).
 - Output projection in bf16 accumulates both head-pairs into ONE
   partial per core; ACT evacuates PSUM directly to bf16 for DMA.
Host sums 8 bf16 partials and adds (bv @ Wo + bo) once.
"""

import sys

import numpy as np

try:
    import concourse.bacc as bacc
except ImportError:  # grading dir may not have the repo on sys.path
    sys.path.insert(0, "/opt/trn_rl_repo")
    import concourse.bacc as bacc

import ml_dtypes
import concourse.mybir as mybir
import concourse.tile as tile
from concourse import bass_utils

B, S, D, H, DH = 2, 2048, 1024, 16, 64
F32 = mybir.dt.float32
R32 = mybir.dt.float32r
F8 = mybir.dt.float8e4
BF16 = mybir.dt.bfloat16
I16 = mybir.dt.int16
EXP = mybir.ActivationFunctionType.Exp
COPY = mybir.ActivationFunctionType.Copy
DR = mybir.MatmulPerfMode.DoubleRow
MUL = mybir.AluOpType.mult
ADD = mybir.AluOpType.add

WS = 32.0                      # host-side weight scale
ESCALE = 0.125 / (WS * WS)     # exp argument scale on raw scores
EBIAS = -4.0                   # exp margin: fp8 weights stay < 240 even at
                               # the heavy product-sum score tails (~8.4 max)
# Schraudolph fast-exp in bf16 bit domain: i16 = x*A + Bc
FE_A = 184.6650292610704 * ESCALE
FE_B = 16256.0 - 5.59 + EBIAS * 184.6650292610704

# exp engine per key-chunk pair (8 pairs/block): True = ACT, False = DVE
ACT_PAIR = [True, False, True, False, True, False, True, True]


def _emit(nc, aps):
    xq, xk, xv = aps["xqT"], aps["xkT"], aps["xvT"]
    out_ap = aps["out"]

    with tile.TileContext(nc) as tc, \
         nc.allow_low_precision(reason="fp8 doublerow pipeline"):
        with tc.tile_pool(name="persist", bufs=1, space="SBUF") as sb, \
             tc.tile_pool(name="xres", bufs=4, space="SBUF") as xvp, \
             tc.tile_pool(name="xstream", bufs=3, space="SBUF") as xp, \
             tc.tile_pool(name="pexp", bufs=4, space="SBUF") as pa_pool, \
             tc.tile_pool(name="zpool", bufs=2, space="SBUF") as z_pool, \
             tc.tile_pool(name="obpool", bufs=4, space="SBUF") as ob_pool:

            wq_sb = sb.tile([128, 2048], F8)
            wk_sb = sb.tile([128, 2048], F8)
            wv_sb = sb.tile([128, 2048], F8)
            wo_sb = sb.tile([128, 2048], BF16)
            bqT_sb = sb.tile([128, 2], F32)
            bkT_sb = sb.tile([128, 2], F32)
            ebias_sb = sb.tile([128, 1], F32)
            ident = sb.tile([128, 128], BF16)   # transpose permutation
            qF8 = sb.tile([128, 5120], F8)      # pad|pairA|pairB|pad
            kf8 = [sb.tile([128, 4096], F8, name=f"kf8_{i}") for i in range(4)]
            vaug = sb.tile([128, 4160], F8)     # [key, chunk*260+head*65+d|1]
            attnT = sb.tile([128, 4096], BF16)  # [pairdims, pair*2048+seq]

            nc.vector.memset(ebias_sb[:], EBIAS)
            onesF = z_pool.tile([128, 64], F32, tag="z", name="onesF")
            nc.vector.memset(onesF[:], 1.0)
            # ones column per (chunk, head) group for the Z sums
            nc.vector.tensor_copy(
                vaug[:].rearrange("p (j g r) -> p j g r", g=4, r=65)
                    [:, :, :, 64:65],
                onesF[:, 0:64].rearrange("p (j g) -> p j g", g=4)
                    .unsqueeze(3))
            nc.sync.dma_start(ident[:], aps["ident"][:])
            # zero the dead regions fp8 DR matmuls rely on (Pool is idle)
            nc.gpsimd.memset(qF8[:, 0:512], 0.0)
            nc.gpsimd.memset(qF8[:, 4608:5120], 0.0)
            for i in range(4):
                nc.gpsimd.memset(kf8[i][:], 0.0)

            nc.sync.dma_start(
                wq_sb[:].rearrange("p (d c) -> p d c", c=256),
                aps["wq"][:].rearrange("(d p) c -> p d c", p=128))
            nc.sync.dma_start(bqT_sb[:], aps["bqT"][:])

            def proj_mm(dst, w_sb, xt, t, sc, first, last):
                for cc in range(2):
                    nc.tensor.matmul(
                        dst[cc * 4 + sc][:],
                        w_sb[:].rearrange("p (d c) -> p d c", c=256)
                            [:, 2 * t:2 * t + 2, cc * 128:cc * 128 + 128],
                        xt[:].rearrange("p (two n) -> p two n", two=2)
                            [:, :, sc * 512:sc * 512 + 512],
                        start=first, stop=last, perf_mode=DR)

            # ---- q / v / k projections share one 8-bank PSUM ring ----
            with tc.tile_pool(name="projp", bufs=8, space="PSUM") as pp:
                qps = [pp.tile([128, 512], F32, tag="pp", name=f"qp{i}")
                       for i in range(8)]
                # dummy matmuls ramp the PE clock while x streams in
                for i in range(16):
                    nc.tensor.matmul(qps[0][0:64, 0:64], onesF[:, 0:64],
                                     onesF[:, 0:64], start=True, stop=True)
                xsrc = {"q": xq, "k": xk, "v": xv}

                def xdram(which, t):
                    return xsrc[which][:].rearrange(
                        "(t i p) n -> t p i n", i=2, p=128)[t]

                for t in range(4):
                    xt = xp.tile([128, 4096], F8, tag="xs", name=f"xq{t}")
                    nc.sync.dma_start(
                        xt[:].rearrange("p (i n) -> p i n", i=2),
                        xdram("q", t))
                    for sc in range(4):
                        proj_mm(qps, wq_sb, xt, t, sc, t == 0, t == 3)
                nc.sync.dma_start(
                    wv_sb[:].rearrange("p (d c) -> p d c", c=256),
                    aps["wv"][:].rearrange("(d p) c -> p d c", p=128))
                xvt = []
                for t in range(4):
                    xt = xvp.tile([128, 4096], F8, tag="xv", name=f"xv{t}")
                    nc.sync.dma_start(
                        xt[:].rearrange("p (i n) -> p i n", i=2),
                        xdram("v", t))
                    xvt.append(xt)
                for cc in range(2):
                    for sc in range(4):
                        nc.vector.tensor_scalar_add(
                            qF8[:, 512 + cc * 2048 + sc * 512:
                                512 + cc * 2048 + sc * 512 + 512],
                            qps[cc * 4 + sc][:], bqT_sb[:, cc:cc + 1])

                # v: t-outer so each resident xv chunk is consumed on arrival
                for half in range(2):
                    vps = [pp.tile([128, 256], F32, tag="pp",
                                   name=f"vp{half}_{i}") for i in range(8)]
                    for t in range(4):
                        for i in range(8):
                            jj = half * 8 + i
                            nc.tensor.matmul(
                                vps[i][:],
                                xvt[t][:].rearrange("p (two n) -> p two n",
                                                    two=2)
                                    [:, :, jj * 128:jj * 128 + 128],
                                wv_sb[:].rearrange("p (d c) -> p d c", c=256)
                                    [:, 2 * t:2 * t + 2, :],
                                start=(t == 0), stop=(t == 3), perf_mode=DR)
                    if half == 0:
                        nc.sync.dma_start(
                            wk_sb[:].rearrange("p (d c) -> p d c", c=256),
                            aps["wk"][:].rearrange("(d p) c -> p d c", p=128))
                        nc.sync.dma_start(bkT_sb[:], aps["bkT"][:])
                    for i in range(8):
                        jj = half * 8 + i
                        nc.vector.tensor_copy(
                            vaug[:, jj * 260:jj * 260 + 260]
                                .rearrange("p (g r) -> p g r", r=65)
                                [:, :, 0:64],
                            vps[i][:].rearrange("p (g d) -> p g d", d=64))

                kps = [pp.tile([128, 512], F32, tag="pp", name=f"kp{i}")
                       for i in range(8)]
                for t in range(4):
                    xt = xp.tile([128, 4096], F8, tag="xs", name=f"xk{t}")
                    nc.sync.dma_start(
                        xt[:].rearrange("p (i n) -> p i n", i=2),
                        xdram("k", t))
                    for sc in range(4):
                        proj_mm(kps, wk_sb, xt, t, sc, t == 0, t == 3)
                for cc in range(2):
                    for sc in range(4):
                        # head-even rows into kf8_A half0, head-odd rows
                        # into kf8_B half1 (dead zones stay zero)
                        nc.vector.tensor_scalar_add(
                            kf8[2 * cc][0:64, sc * 512:sc * 512 + 512],
                            kps[cc * 4 + sc][0:64, :], bkT_sb[0:64, cc:cc + 1])
                        nc.vector.tensor_scalar_add(
                            kf8[2 * cc + 1][64:128,
                                            2048 + sc * 512:2048 + sc * 512 + 512],
                            kps[cc * 4 + sc][64:128, :],
                            bkT_sb[64:128, cc:cc + 1])
                nc.sync.dma_start(
                    wo_sb[:].rearrange("p (d c) -> p d c", c=1024),
                    aps["wo"][:].rearrange("(d p) c -> p d c", p=128))

            # ---- attention + normalization + output projection ----
            with tc.tile_pool(name="sp", bufs=2, space="PSUM") as sp, \
                 tc.tile_pool(name="acc", bufs=2, space="PSUM") as accp:

                def norm(p, qoff, qlen, acc, bi):
                    nq = qlen // 128
                    attnN = z_pool.tile([128, 512], BF16, tag="z",
                                        name=f"an{bi}")
                    tsp = sp.tile([128, 512], BF16, tag="s", name=f"tp{bi}")
                    for h2 in range(2):
                        zr = z_pool.tile([128, 4], F32, tag="zr",
                                         name=f"zr{bi}_{h2}")
                        av = acc[h2][:, 0:nq * 65] \
                            .rearrange("p (q r) -> p q r", r=65)
                        nc.vector.reciprocal(zr[:, 0:nq], av[:, :, 64:65])
                        nc.vector.tensor_tensor(
                            attnN[:, h2 * 256:h2 * 256 + nq * 64]
                                .rearrange("p (q d) -> p q d", d=64),
                            av[:, :, 0:64],
                            zr[:, 0:nq].unsqueeze(2)
                                .broadcast_to([128, nq, 64]),
                            MUL)
                        for qc in range(nq):
                            nc.tensor.matmul(
                                tsp[h2 * 64:h2 * 64 + 64,
                                    qc * 128:qc * 128 + 128],
                                attnN[:, h2 * 256 + qc * 64:
                                      h2 * 256 + qc * 64 + 64],
                                ident[:], is_transpose=True)
                    nc.vector.tensor_copy(
                        attnT[:, p * 2048 + qoff:p * 2048 + qoff + qlen],
                        tsp[:, 0:qlen])

                def outproj(qoff, qlen, bi):
                    for tq in range(qoff // 128, (qoff + qlen) // 128):
                        up = sp.tile([128, 1024], F32, tag="s",
                                     name=f"u{bi}_{tq}")
                        for n in range(2):
                            for p in range(2):
                                nc.tensor.matmul(
                                    up[:, n * 512:n * 512 + 512],
                                    attnT[:, p * 2048 + tq * 128:
                                          p * 2048 + tq * 128 + 128],
                                    wo_sb[:, p * 1024 + n * 512:
                                          p * 1024 + n * 512 + 512],
                                    start=(p == 0), stop=(p == 1))
                        ob = ob_pool.tile([128, 1024], BF16, tag="ob",
                                          name=f"ob{bi}_{tq}")
                        nc.scalar.activation(ob[:], up[:], COPY)
                        nc.sync.dma_start(
                            out_ap[tq * 128:tq * 128 + 128, :], ob[:])

                def qview(p, g2, qoff, qlen):
                    base = 512 + p * 2048 + qoff - g2 * 512
                    return qF8[:, base:base + 1024] \
                        .rearrange("p (two n) -> p two n", two=2)[:, :, 0:qlen]

                def kview(p, g2, j):
                    return kf8[2 * p + g2][:] \
                        .rearrange("p (two m) -> p two m", two=2) \
                        [:, :, j * 128:j * 128 + 128]

                def vview(t, p, g2):
                    return vaug[:].rearrange("p (j g r) -> p j g r",
                                             g=4, r=65) \
                        [:, 2 * t:2 * t + 2, 2 * p + g2, :]

                def pqview(pB, g2, qc):
                    return pB[:].rearrange("p (two n) -> p two n", two=2) \
                        [:, :, g2 * 512 + qc * 128:g2 * 512 + qc * 128 + 128]

                blocks = [(p, Q * 512, 512) for Q in range(4) for p in range(2)]
                pend_norm = None
                pend_out = None
                for bi, (p, qoff, qlen) in enumerate(blocks):
                    acc = [accp.tile([128, 260], F32, tag="acc",
                                     name=f"acc{bi}_{h2}") for h2 in range(2)]
                    pBs = [None] * 8

                    def attnv(t, p=p, qlen=qlen, pBs=pBs, acc=acc):
                        # one accumulation group per acc tile (= one PSUM
                        # zero region): start only on the very first write
                        for g2 in range(2):
                            for qc in range(qlen // 128):
                                nc.tensor.matmul(
                                    acc[g2][:, qc * 65:qc * 65 + 65],
                                    pqview(pBs[t], g2, qc),
                                    vview(t, p, g2),
                                    start=(t == 0 and qc == 0),
                                    stop=(t == 7 and qc == qlen // 128 - 1),
                                    perf_mode=DR)

                    for t in range(8):
                        sBigs = []
                        for i in range(2):
                            j = 2 * t + i
                            sBig = sp.tile([128, 1024], F32, tag="s",
                                           name=f"s{bi}_{j}")
                            for g2 in range(2):
                                nc.tensor.matmul(
                                    sBig[:, g2 * 512:g2 * 512 + qlen],
                                    kview(p, g2, j), qview(p, g2, qoff, qlen),
                                    start=True, stop=True, perf_mode=DR)
                            sBigs.append(sBig)
                        if ACT_PAIR[t]:
                            pB = pa_pool.tile([128, 2048], F8, tag="pa",
                                              name=f"pb{bi}_{t}")
                            for i in range(2):
                                if qlen == 512:
                                    nc.scalar.activation(
                                        pB[:, i * 1024:i * 1024 + 1024],
                                        sBigs[i][:], EXP, scale=ESCALE,
                                        bias=ebias_sb[:])
                                else:
                                    sv = sBigs[i][:].rearrange(
                                        "p (g c) -> p g c", c=512)[:, :, 0:qlen]
                                    pv = pB[:, i * 1024:i * 1024 + 1024] \
                                        .rearrange("p (g c) -> p g c",
                                                   c=512)[:, :, 0:qlen]
                                    nc.scalar.activation(pv, sv, EXP,
                                                         scale=ESCALE,
                                                         bias=ebias_sb[:])
                        else:
                            pI = pa_pool.tile([128, 2048], I16, tag="pa",
                                              name=f"pi{bi}_{t}")
                            for i in range(2):
                                if qlen == 512:
                                    nc.vector.tensor_scalar(
                                        pI[:, i * 1024:i * 1024 + 1024],
                                        sBigs[i][:], FE_A, FE_B, MUL, ADD)
                                else:
                                    sv = sBigs[i][:].rearrange(
                                        "p (g c) -> p g c", c=512)[:, :, 0:qlen]
                                    pv = pI[:, i * 1024:i * 1024 + 1024] \
                                        .rearrange("p (g c) -> p g c",
                                                   c=512)[:, :, 0:qlen]
                                    nc.vector.tensor_scalar(pv, sv, FE_A,
                                                            FE_B, MUL, ADD)
                            pB = pa_pool.tile([128, 2048], F8, tag="pa",
                                              name=f"pb{bi}_{t}")
                            nc.gpsimd.tensor_copy(pB[:], pI[:].bitcast(BF16))
                        pBs[t] = pB
                        if t == 1 and pend_norm is not None:
                            pend_norm()
                            pend_norm = None
                        if t >= 1:
                            attnv(t - 1)
                        if t == 3 and pend_out is not None:
                            pend_out()
                            pend_out = None
                    attnv(7)
                    pend_norm = (lambda p=p, qoff=qoff, qlen=qlen,
                                 acc=acc, bi=bi:
                                 norm(p, qoff, qlen, acc, bi))
                    if p == 1:
                        prev = pend_out
                        pend_out = (lambda qoff=qoff, qlen=qlen, bi=bi:
                                    outproj(qoff, qlen, bi))
                        if prev is not None:
                            prev()
                pend_norm()
                pend_out()


_NC = None


def _get_nc():
    global _NC
    if _NC is None:
        nc = bacc.Bacc("TRN2", target_bir_lowering=False, debug=False,
                       enable_asserts=False, num_devices=8)
        aps = {}
        for nm in ("xqT", "xkT", "xvT"):
            aps[nm] = nc.dram_tensor(nm, (D, S), F8, kind="ExternalInput").ap()
        for nm in ("wq", "wk", "wv"):
            aps[nm] = nc.dram_tensor(nm, (D, 256), F8,
                                     kind="ExternalInput").ap()
        aps["wo"] = nc.dram_tensor("wo", (256, D), BF16,
                                   kind="ExternalInput").ap()
        aps["ident"] = nc.dram_tensor("ident", (128, 128), BF16,
                                      kind="ExternalInput").ap()
        for nm in ("bqT", "bkT"):
            aps[nm] = nc.dram_tensor(nm, (128, 2), F32,
                                     kind="ExternalInput").ap()
        aps["out"] = nc.dram_tensor("out", (S, D), BF16,
                                    kind="ExternalOutput").ap()
        _emit(nc, aps)
        nc.compile()
        _NC = nc
    return _NC


def _run(inputs, trace=False):
    nc = _get_nc()
    f = np.float32
    e4 = ml_dtypes.float8_e4m3
    bf = ml_dtypes.bfloat16
    q = np.asarray(inputs["query"], dtype=f)
    k = np.asarray(inputs["key"], dtype=f)
    v = np.asarray(inputs["value"], dtype=f)
    Wq = np.asarray(inputs["Wq"], dtype=f)
    Wk = np.asarray(inputs["Wk"], dtype=f)
    Wv = np.asarray(inputs["Wv"], dtype=f)
    Wo = np.asarray(inputs["Wo"], dtype=f)
    bq = np.asarray(inputs["bq"], dtype=f)
    bk = np.asarray(inputs["bk"], dtype=f)
    bv = np.asarray(inputs["bv"], dtype=f)
    bo = np.asarray(inputs["bo"], dtype=f)

    xT = {b: (np.ascontiguousarray(q[b].T).astype(e4),
              np.ascontiguousarray(k[b].T).astype(e4),
              np.ascontiguousarray(v[b].T).astype(e4)) for b in range(B)}
    in_maps = []
    for i in range(8):
        b, hg = divmod(i, 4)
        c0 = hg * 256
        in_maps.append({
            "xqT": xT[b][0], "xkT": xT[b][1], "xvT": xT[b][2],
            "wq": np.ascontiguousarray(Wq[:, c0:c0 + 256] * WS).astype(e4),
            "wk": np.ascontiguousarray(Wk[:, c0:c0 + 256] * WS).astype(e4),
            "wv": np.ascontiguousarray(Wv[:, c0:c0 + 256] * WS).astype(e4),
            "bqT": np.ascontiguousarray(
                (bq[c0:c0 + 256] * WS).reshape(2, 128).T),
            "bkT": np.ascontiguousarray(
                (bk[c0:c0 + 256] * WS).reshape(2, 128).T),
            "wo": np.ascontiguousarray(Wo[c0:c0 + 256, :] / WS).astype(bf),
            "ident": np.eye(128, dtype=np.float32).astype(bf),
        })

    res = bass_utils.run_bass_kernel_spmd(nc, in_maps, core_ids=list(range(8)),
                                          trace=trace)
    out = np.zeros((B, S, D), dtype=f)
    for i in range(8):
        out[i // 4] += np.asarray(res.results[i]["out"]).astype(f)
    out += (bv @ Wo + bo)[None, None, :]
    return out, res


def kernel(**inputs):
    out, _ = _run(inputs, trace=False)
    return out
